# revision 7
# baseline (speedup 1.0000x reference)
"""GateLoop (B=4, N=4096, D=1024) Trainium2 kernel over 8 NeuronCores.

Sharding: data-parallel over the 4 batch elements x 2-way tensor-parallel
split of the D=1024 recurrence channels (the complex diagonal recurrence is
independent per channel). Core c handles batch c//2, channels
[(c%2)*512 : (c%2+1)*512]. Each core computes its projections, runs the
scan over the full sequence for its 512 channels, and produces a partial
y @ wo[ch, :] of shape (1024, 4096) (transposed). The host sums the two
partials per batch and transposes back. No cross-core communication.

Scan formulation (avoids complex arithmetic + overflow): with
a_t = m_t * cis(phi_t), m_t = sigmoid(|a_t|), theta_t = arctan(ai/ar)
in (-pi/2, pi/2) (SIGNED division so the ar<0 half-plane flip folds into
the signed multiplier mt_t = m_t * sign(ar_t)). With Theta_t =
cumsum(theta) the recurrence becomes two independent REAL first-order
scans
    Zr_t = mt_t * Zr_{t-1} + kv_t * cos(Theta_t)
    Zi_t = mt_t * Zi_{t-1} + kv_t * sin(Theta_t)
and Re(S_t) = cos(Theta_t) * Zr_t + sin(Theta_t) * Zi_t, which map onto
the DVE TensorTensorScan instruction (fp32 state, |mt| < 1 so stable).
The Theta scan re-bases each block from the range-reduced thr endpoint
(equivalent mod 2pi, keeps Theta < ~810 in fp32); sigmoid is synthesized
as 0.5*(1+tanh(r/2)) so it lands in the silu table set.

Schedule: 2-block SUPERSTEPS. The per-block phase chain (drain -> sqrt ->
arctan -> cumsum -> sin -> scans -> out-proj input) is a ~55 us latency
loop; at depth 1 it paces the kernel at ~70+ us/block even though no
engine is that busy. Grouping two blocks per activation-table rotation
(A(b0) A(b1) | B(b0) B(b1) | C(b0) C(b1)) gives every cross-engine edge a
block of slack, so the 96.4 us of PE work per superstep paces instead.
  Phase A [sqrt set]    : PSUM drains (squares/sign/copies) + Sqrt
  Phase B [trig set]    : Arctan + DVE cumsum/range-reduce + both Sins
  Phase C [silu set]    : Silu + Tanh (sigmoid synth); DVE mt/scans; Pool
                          products; out-proj feeds
Out-projection of block j runs as a 32-matmul burst after proj(j+2), two
blocks after its inputs were produced. Weights DMA in consumption order
(wq, xb0, wk, wv, wg, war, wai, xb1, wo) on the FIFO SP queue.
"""
import math
import os

import numpy as np
import ml_dtypes

B, N, D = 4, 4096, 1024
CH = 512            # channels per core (tensor-parallel half)
NCG = CH // 128     # 4 channel groups of 128 partitions
T = 512             # token block
NBLK = N // T
NSUP = NBLK // 2
P = 128
KT = D // P         # contraction tiles
EPS = 1e-5
BF16 = ml_dtypes.bfloat16

TWO_PI = 2 * math.pi
C1 = float(np.float32(6.28125))
C2 = float(np.float32(np.float64(TWO_PI) - 6.28125))
C3 = float(np.float32(np.float64(TWO_PI) - 6.28125
                      - np.float64(np.float32(np.float64(TWO_PI) - 6.28125))))
MAGIC = float(np.float32(1.5 * 2 ** 23))
INV2PI = float(np.float32(1.0 / TWO_PI))
PI = float(np.float32(math.pi))
PIH = float(np.float32(math.pi / 2))

_NC = None
LAST_RESULT = None  # BassKernelResults of the most recent run (for profiling)


def _build():
    from contextlib import ExitStack
    from concourse import bacc
    import concourse.mybir as mybir
    import concourse.tile as tile
    from concourse.mybir import ActivationFunctionType as AF, AluOpType as OP

    fp32 = mybir.dt.float32
    bf = mybir.dt.bfloat16

    nc = bacc.Bacc(None, target_bir_lowering=False)

    xnT_d = nc.dram_tensor("xnT", [D, N], bf, kind="ExternalInput")
    wnames = ["wq", "wk", "wv", "wg", "war", "wai"]
    w_d = {n: nc.dram_tensor(n, [D, CH], bf, kind="ExternalInput") for n in wnames}
    wo_d = nc.dram_tensor("wo", [CH, D], bf, kind="ExternalInput")
    outT_d = nc.dram_tensor("outT", [D, N], bf, kind="ExternalOutput")

    xnT_t = xnT_d.rearrange("(ko p) n -> p ko n", p=P)
    outT_t = outT_d.rearrange("(mo p) n -> p mo n", p=P)

    with tile.TileContext(nc) as tc, ExitStack() as ctx:
        wpool = ctx.enter_context(tc.tile_pool(name="w", bufs=1))
        xpool = ctx.enter_context(tc.tile_pool(name="x", bufs=2))
        cpool = ctx.enter_context(tc.tile_pool(name="c", bufs=2))   # cross-phase, per block
        kpool = ctx.enter_context(tc.tile_pool(name="k", bufs=2))   # cross-block (scan carries)
        scr = ctx.enter_context(tc.tile_pool(name="s", bufs=8))     # fp32 scratch
        sbb = ctx.enter_context(tc.tile_pool(name="sb", bufs=12))   # bf16 scratch
        ypool = ctx.enter_context(tc.tile_pool(name="y", bufs=2))
        obp = ctx.enter_context(tc.tile_pool(name="o", bufs=2))
        gpool = ctx.enter_context(tc.tile_pool(name="g", bufs=2))
        pproj = ctx.enter_context(tc.tile_pool(name="pp", bufs=6, space="PSUM"))
        pout = ctx.enter_context(tc.tile_pool(name="po", bufs=2, space="PSUM"))

        # DMA in consumption order on the FIFO SP queue: wq (first matmul)
        # then xb0, then the remaining proj weights, xb1, wo (out-proj only
        # runs from superstep 1).
        wsb = {}
        wsb["wq"] = wpool.tile([P, KT, CH], bf, tag="w_wq", name="w_wq")
        nc.sync.dma_start(wsb["wq"][:], w_d["wq"].rearrange("(ko p) m -> p ko m", p=P))
        xbs = [None] * NBLK
        xbs[0] = xpool.tile([P, KT, T], bf, tag="xb", name="xb_0")
        nc.sync.dma_start(xbs[0][:], xnT_t[:, :, 0:T])
        for n in ["wk", "wv", "wg", "war", "wai"]:
            t_ = wpool.tile([P, KT, CH], bf, tag=f"w_{n}")
            nc.sync.dma_start(t_[:], w_d[n].rearrange("(ko p) m -> p ko m", p=P))
            wsb[n] = t_
        xbs[1] = xpool.tile([P, KT, T], bf, tag="xb", name="xb_1")
        nc.sync.dma_start(xbs[1][:], xnT_t[:, :, T:2 * T])
        wosb = wpool.tile([P, CH // P, D], bf, tag="w_wo")
        nc.sync.dma_start(wosb[:], wo_d.rearrange("(ko p) m -> p ko m", p=P))

        negmagic = wpool.tile([P, T], fp32, tag="negmagic", name="negmagic")
        nc.gpsimd.memset(negmagic[:], -MAGIC)

        prevThc = [None] * NCG   # [P,1] carry of the reduced Theta endpoint
        prevZr = [None] * NCG
        prevZi = [None] * NCG
        ys_all = [None] * NBLK
        gC = 0.0  # cross-super act-phase gate

        # Activation-table discipline: Square/Sign/Copy live in every set;
        # the set-specific phases are Sqrt -> {Arctan,Sin} -> {Silu,Tanh}.
        # Zero-valued [P,1] gate tiles, fed as activation bias (adds 0),
        # pin each set-specific phase after the previous one so the eager
        # Tile scheduler cannot interleave phases and thrash the 1.3us
        # table loads: 3-4 loads per 2-block superstep.

        def emit_proj(blk):
            xb = xbs[blk]
            PS = [None] * NCG
            for cg in range(NCG):
                cs = slice(cg * P, (cg + 1) * P)
                ps = {}
                for n in wnames:
                    pt = pproj.tile([P, T], fp32, tag="proj")
                    for k in range(KT):
                        nc.tensor.matmul(pt[:], wsb[n][:, k, cs], xb[:, k, :],
                                         start=(k == 0), stop=(k == KT - 1))
                    ps[n] = pt
                PS[cg] = ps
            return PS

        def emit_outproj(blk):
            ys = ys_all[blk]
            t0 = blk * T
            for mo in range(D // P):
                pso = pout.tile([P, T], fp32, tag="out")
                for cg in range(NCG):
                    nc.tensor.matmul(pso[:], wosb[:, cg, mo * P:(mo + 1) * P],
                                     ys[cg][:], start=(cg == 0), stop=(cg == NCG - 1))
                ob = obp.tile([P, T], bf, tag="ob")
                # Pool has no PSUM port; alternate the evacuation between
                # Act and DVE explicitly.
                if mo % 2 == 0:
                    nc.scalar.copy(ob[:], pso[:])
                else:
                    nc.vector.tensor_copy(ob[:], pso[:])
                nc.sync.dma_start(outT_t[:, mo, t0:t0 + T], ob[:])

        for sup in range(NSUP):
            b0, b1 = 2 * sup, 2 * sup + 1
            if b0 + 2 < NBLK:
                xbs[b0 + 2] = xpool.tile([P, KT, T], bf, tag="xb",
                                         name=f"xb_{b0 + 2}")
                nc.sync.dma_start(xbs[b0 + 2][:],
                                  xnT_t[:, :, (b0 + 2) * T:(b0 + 3) * T])
            PS = {}
            PS[b0] = emit_proj(b0)
            if b0 >= 2:
                emit_outproj(b0 - 2)
            if b1 + 2 < NBLK:
                xbs[b1 + 2] = xpool.tile([P, KT, T], bf, tag="xb",
                                         name=f"xb_{b1 + 2}")
                nc.sync.dma_start(xbs[b1 + 2][:],
                                  xnT_t[:, :, (b1 + 2) * T:(b1 + 3) * T])
            PS[b1] = emit_proj(b1)
            if b1 >= 2:
                emit_outproj(b1 - 2)

            # --- phase A: set-free PSUM drains + Sqrt (sqrt set) --------
            kv = {}; qs = {}; gb = {}; sgn = {}; ratio = {}; r = {}
            for b in (b0, b1):
                ps_all = PS[b]
                for cg in range(NCG):
                    ps = ps_all[cg]
                    sq1 = scr.tile([P, T], fp32, tag="scr")
                    nc.scalar.square(sq1[:], ps["war"][:])
                    sq2 = scr.tile([P, T], fp32, tag="scr")
                    nc.scalar.square(sq2[:], ps["wai"][:])
                    sgn[b, cg] = cpool.tile([P, T], bf, tag=f"sgn{cg}",
                                            name=f"sgn{cg}_{b}")
                    nc.scalar.sign(sgn[b, cg][:], ps["war"][:])
                    rec = scr.tile([P, T], fp32, tag="scr")
                    nc.vector.reciprocal_approx_fast(rec[:], ps["war"][:])
                    rt = scr.tile([P, T], fp32, tag="scr")
                    nc.vector.tensor_tensor(rt[:], ps["wai"][:], rec[:], OP.mult)
                    # clamp: the HW arctan table misbehaves for huge |x|
                    # (1/ar is unbounded); arctan(1e4) ~ pi/2 - 1e-4
                    ratio[b, cg] = cpool.tile([P, T], bf, tag=f"ratio{cg}",
                                              name=f"ratio{cg}_{b}")
                    nc.vector.tensor_scalar(ratio[b, cg][:], rt[:], 1e4, -1e4,
                                            OP.min, OP.max)
                    vs = sbb.tile([P, T], bf, tag="sbb", name=f"vs{cg}_{b}")
                    nc.scalar.copy(vs[:], ps["wv"][:])
                    kv[b, cg] = cpool.tile([P, T], bf, tag=f"kv{cg}",
                                           name=f"kv{cg}_{b}")
                    nc.vector.tensor_tensor(kv[b, cg][:], ps["wk"][:], vs[:],
                                            OP.mult)
                    qs[b, cg] = cpool.tile([P, T], bf, tag=f"qs{cg}",
                                           name=f"qs{cg}_{b}")
                    nc.scalar.copy(qs[b, cg][:], ps["wq"][:])
                    gb[b, cg] = cpool.tile([P, T], bf, tag=f"gb{cg}",
                                           name=f"gb{cg}_{b}")
                    nc.scalar.copy(gb[b, cg][:], ps["wg"][:])
                    r2 = scr.tile([P, T], fp32, tag="scr")
                    nc.gpsimd.tensor_tensor(r2[:], sq1[:], sq2[:], OP.add)
                    r[b, cg] = cpool.tile([P, T], bf, tag=f"r{cg}",
                                          name=f"r{cg}_{b}")
                    nc.scalar.activation(r[b, cg][:], r2[:], AF.Sqrt, bias=gC)

            # gate A: arctan/sin phase waits for the last Sqrt
            gA = gpool.tile([P, 1], fp32, tag="gA", name=f"gA_{sup}")
            nc.vector.tensor_scalar(gA[:], r[b1, NCG - 1][:, 0:1], 0.0, None,
                                    OP.mult)

            # --- phase B (trig set): Arctan + cumsum/range-reduce + Sins
            ui = {}; ur = {}
            for b in (b0, b1):
                thrs = [None] * NCG
                thcs = [None] * NCG
                for cg in range(NCG):
                    th = scr.tile([P, T], fp32, tag="scr")
                    nc.scalar.activation(th[:], ratio[b, cg][:], AF.Arctan,
                                         bias=gA[:, 0:1])
                    Th = scr.tile([P, T], fp32, tag="scr")
                    init = 0.0 if b == 0 else prevThc[cg][:, 0:1]
                    nc.vector.tensor_tensor_scan(Th[:], th[:], th[:],
                                                 init, OP.add, OP.bypass)
                    k2 = scr.tile([P, T], fp32, tag="scr")
                    nc.vector.affine_then_add(k2[:], Th[:], negmagic[:],
                                              INV2PI, MAGIC)
                    thr = scr.tile([P, T], fp32, tag="scr")
                    nc.vector.cody_waite_cascade(thr[:], Th[:], k2[:], C1, C2, C3)
                    thc = scr.tile([P, T], fp32, tag="scr")
                    nc.vector.add_range_wrap(thc[:], thr[:], PIH, PI,
                                             float(np.float32(TWO_PI)))
                    tc_ = kpool.tile([P, 1], fp32, tag=f"thc{cg}",
                                     name=f"thcar{cg}_{b}")
                    nc.vector.tensor_scalar(tc_[:], thr[:, T - 1:T], 0.0, None,
                                            OP.add)
                    prevThc[cg] = tc_
                    thrs[cg], thcs[cg] = thr, thc
                for cg in range(NCG):
                    ui[b, cg] = cpool.tile([P, T], bf, tag=f"ui{cg}",
                                           name=f"ui{cg}_{b}")
                    nc.scalar.activation(ui[b, cg][:], thrs[cg][:], AF.Sin,
                                         bias=gA[:, 0:1])
                    ur[b, cg] = cpool.tile([P, T], bf, tag=f"ur{cg}",
                                           name=f"ur{cg}_{b}")
                    nc.scalar.activation(ur[b, cg][:], thcs[cg][:], AF.Sin,
                                         bias=gA[:, 0:1])

            # gate B: silu/tanh phase waits for the last Sins
            gB = gpool.tile([P, 1], fp32, tag="gB", name=f"gB_{sup}")
            nc.vector.scalar_tensor_tensor(gB[:], ui[b1, NCG - 1][:, 0:1], 0.0,
                                           ur[b1, NCG - 1][:, 0:1],
                                           OP.mult, OP.mult)

            # --- phase C (silu set): Silu + Tanh (sigmoid synth), scans,
            # recombination. y = (q*silu(g)) * re is folded as qsg = q*sg so
            # the final product is one DVE op; y1(cg) is emitted after
            # cg+1's scans so DVE never waits on the Pool chain. ----------
            tms = None
            for b in (b0, b1):
                ys = [None] * NCG
                res = [None] * NCG
                qsg = [None] * NCG

                def emit_y1(cg, b=b, ys=ys, res=res, qsg=qsg):
                    y1 = ypool.tile([P, T], bf, tag=f"y{cg}", name=f"y{cg}_{b}")
                    nc.vector.tensor_tensor(y1[:], qsg[cg][:], res[cg][:],
                                            OP.mult)
                    ys[cg] = y1

                for cg in range(NCG):
                    sg = sbb.tile([P, T], bf, tag="sbb", name=f"sg{cg}_{b}")
                    nc.scalar.activation(sg[:], gb[b, cg][:], AF.Silu,
                                         bias=gB[:, 0:1])
                    tm = sbb.tile([P, T], bf, tag="sbb", name=f"tm{cg}_{b}")
                    nc.scalar.activation(tm[:], r[b, cg][:], AF.Tanh,
                                         bias=gB[:, 0:1], scale=0.5)
                    tms = tm
                    qsg[cg] = cpool.tile([P, T], bf, tag=f"qsg{cg}",
                                         name=f"qsg{cg}_{b}")
                    nc.gpsimd.tensor_tensor(qsg[cg][:], qs[b, cg][:], sg[:],
                                            OP.mult)
                    # sigmoid(r) = 0.5*(1+tanh(r/2)); fold the sign in next
                    m = sbb.tile([P, T], bf, tag="sbb", name=f"m{cg}_{b}")
                    nc.vector.tensor_scalar(m[:], tm[:], 1.0, 0.5,
                                            OP.add, OP.mult)
                    mt = sbb.tile([P, T], bf, tag="sbb", name=f"mt{cg}_{b}")
                    nc.vector.tensor_tensor(mt[:], m[:], sgn[b, cg][:], OP.mult)
                    wr = sbb.tile([P, T], bf, tag="sbb", name=f"wr{cg}_{b}")
                    nc.vector.tensor_tensor(wr[:], kv[b, cg][:], ur[b, cg][:],
                                            OP.mult)
                    wi = sbb.tile([P, T], bf, tag="sbb", name=f"wi{cg}_{b}")
                    nc.vector.tensor_tensor(wi[:], kv[b, cg][:], ui[b, cg][:],
                                            OP.mult)
                    Zr = kpool.tile([P, T], bf, tag=f"Zr{cg}")
                    initr = 0.0 if b == 0 else prevZr[cg][:, T - 1:T]
                    nc.vector.tensor_tensor_scan(Zr[:], mt[:], wr[:], initr,
                                                 OP.mult, OP.add)
                    Zi = kpool.tile([P, T], bf, tag=f"Zi{cg}")
                    initi = 0.0 if b == 0 else prevZi[cg][:, T - 1:T]
                    nc.vector.tensor_tensor_scan(Zi[:], mt[:], wi[:], initi,
                                                 OP.mult, OP.add)
                    if cg > 0:
                        emit_y1(cg - 1)
                    t1 = sbb.tile([P, T], bf, tag="sbb", name=f"t1{cg}_{b}")
                    nc.gpsimd.tensor_tensor(t1[:], ur[b, cg][:], Zr[:], OP.mult)
                    t2 = sbb.tile([P, T], bf, tag="sbb", name=f"t2{cg}_{b}")
                    nc.gpsimd.tensor_tensor(t2[:], ui[b, cg][:], Zi[:], OP.mult)
                    re = sbb.tile([P, T], bf, tag="sbb", name=f"re{cg}_{b}")
                    nc.gpsimd.tensor_tensor(re[:], t1[:], t2[:], OP.add)
                    res[cg] = re
                    prevZr[cg], prevZi[cg] = Zr, Zi
                emit_y1(NCG - 1)
                ys_all[b] = ys

            # gate C: next superstep's Sqrt waits for the last Tanh
            gCt = gpool.tile([P, 1], fp32, tag="gC", name=f"gC_{sup}")
            nc.vector.tensor_scalar(gCt[:], tms[:, 0:1], 0.0, None, OP.mult)
            gC = gCt[:, 0:1]

        emit_outproj(NBLK - 2)
        emit_outproj(NBLK - 1)

    nc.finalize()
    return nc


def _get_nc():
    global _NC
    if _NC is None:
        _NC = _build()
    return _NC


def kernel(**inputs):
    global LAST_RESULT
    from concourse.bass_utils import run_bass_kernel_spmd

    x = np.asarray(inputs["x"], np.float32)
    gamma = np.asarray(inputs["gamma"], np.float32)
    wq = np.asarray(inputs["wq"], np.float32)
    wk = np.asarray(inputs["wk"], np.float32)
    wv = np.asarray(inputs["wv"], np.float32)
    wa = np.asarray(inputs["wa"], np.float32)
    wg = np.asarray(inputs["wg"], np.float32)
    wo = np.asarray(inputs["wo"], np.float32)

    inv = 1.0 / np.sqrt((x * x).sum(-1, keepdims=True) + np.float32(EPS))
    xn = (inv * x * gamma * np.float32(math.sqrt(D))).astype(np.float32)
    xnT = np.ascontiguousarray(xn.transpose(0, 2, 1)).astype(BF16)  # (B, D, N)

    in_maps = []
    for core in range(8):
        b, h = core // 2, core % 2
        ch = slice(h * CH, (h + 1) * CH)
        in_maps.append({
            "xnT": xnT[b],
            "wq": np.ascontiguousarray(wq[:, ch]).astype(BF16),
            "wk": np.ascontiguousarray(wk[:, ch]).astype(BF16),
            "wv": np.ascontiguousarray(wv[:, ch]).astype(BF16),
            "wg": np.ascontiguousarray(wg[:, ch]).astype(BF16),
            "war": np.ascontiguousarray(wa[:, h * CH:(h + 1) * CH]).astype(BF16),
            "wai": np.ascontiguousarray(wa[:, D + h * CH:D + (h + 1) * CH]).astype(BF16),
            "wo": np.ascontiguousarray(wo[ch, :]).astype(BF16),
        })

    nc = _get_nc()
    trace = bool(int(os.environ.get("GATELOOP_TRACE", "0")))
    LAST_RESULT = run_bass_kernel_spmd(
        nc, in_maps, core_ids=list(range(8)), trace=trace,
        trace_cores=list(range(8)) if trace else None,
    )
    res = LAST_RESULT.results

    out = np.empty((B, N, D), np.float32)
    for b in range(B):
        acc = (res[2 * b]["outT"].astype(np.float32)
               + res[2 * b + 1]["outT"].astype(np.float32))   # (D, N)
        out[b] = acc.T
    return out


# revision 8
# speedup vs baseline: 1.1467x; 1.1467x over previous
"""GateLoop (B=4, N=4096, D=1024) Trainium2 kernel over 8 NeuronCores.

Sharding: data-parallel over the 4 batch elements x 2-way tensor-parallel
split of the D=1024 recurrence channels (the complex diagonal recurrence is
independent per channel). Core c handles batch c//2, channels
[(c%2)*512 : (c%2+1)*512]. Each core computes its projections, runs the
scan over the full sequence for its 512 channels, and produces a partial
y @ wo[ch, :] of shape (1024, 4096) (transposed). The host sums the two
partials per batch and transposes back. No cross-core communication.

Scan formulation (avoids complex arithmetic + overflow): with
a_t = m_t * cis(phi_t), m_t = sigmoid(|a_t|), theta_t = arctan(ai/ar)
in (-pi/2, pi/2) (SIGNED division so the ar<0 half-plane flip folds into
the signed multiplier mt_t = m_t * sign(ar_t)). With Theta_t =
cumsum(theta) the recurrence becomes two independent REAL first-order
scans
    Zr_t = mt_t * Zr_{t-1} + kv_t * cos(Theta_t)
    Zi_t = mt_t * Zi_{t-1} + kv_t * sin(Theta_t)
and Re(S_t) = cos(Theta_t) * Zr_t + sin(Theta_t) * Zi_t, which map onto
the DVE TensorTensorScan instruction (fp32 state, |mt| < 1 so stable).
The Theta scan re-bases each block from the range-reduced thr endpoint
(equivalent mod 2pi, keeps Theta < ~810 in fp32); sigmoid is synthesized
as 0.5*(1+tanh(r/2)) so it lands in the silu table set.

Schedule: 2-block SUPERSTEPS. The per-block phase chain (drain -> sqrt ->
arctan -> cumsum -> sin -> scans -> out-proj input) is a ~55 us latency
loop; at depth 1 it paces the kernel at ~70+ us/block even though no
engine is that busy. Grouping two blocks per activation-table rotation
(A(b0) A(b1) | B(b0) B(b1) | C(b0) C(b1)) gives every cross-engine edge a
block of slack, so the 96.4 us of PE work per superstep paces instead.
  Phase A [sqrt set]    : PSUM drains (squares/sign/copies) + Sqrt
  Phase B [trig set]    : Arctan + DVE cumsum/range-reduce + both Sins
  Phase C [silu set]    : Silu + Tanh (sigmoid synth); DVE mt/scans; Pool
                          products; out-proj feeds
Out-projection of block j runs as a 32-matmul burst after proj(j+2), two
blocks after its inputs were produced. Weights DMA in consumption order
(wq, xb0, wk, wv, wg, war, wai, xb1, wo) on the FIFO SP queue.
"""
import math
import os

import numpy as np
import ml_dtypes

B, N, D = 4, 4096, 1024
CH = 512            # channels per core (tensor-parallel half)
NCG = CH // 128     # 4 channel groups of 128 partitions
T = 512             # token block
NBLK = N // T
NSUP = NBLK // 2
P = 128
KT = D // P         # contraction tiles
EPS = 1e-5
BF16 = ml_dtypes.bfloat16

TWO_PI = 2 * math.pi
C1 = float(np.float32(6.28125))
C2 = float(np.float32(np.float64(TWO_PI) - 6.28125))
C3 = float(np.float32(np.float64(TWO_PI) - 6.28125
                      - np.float64(np.float32(np.float64(TWO_PI) - 6.28125))))
MAGIC = float(np.float32(1.5 * 2 ** 23))
INV2PI = float(np.float32(1.0 / TWO_PI))
PI = float(np.float32(math.pi))
PIH = float(np.float32(math.pi / 2))

_NC = None
LAST_RESULT = None  # BassKernelResults of the most recent run (for profiling)


def _build():
    from contextlib import ExitStack
    from concourse import bacc
    import concourse.mybir as mybir
    import concourse.tile as tile
    from concourse.mybir import ActivationFunctionType as AF, AluOpType as OP

    fp32 = mybir.dt.float32
    bf = mybir.dt.bfloat16

    nc = bacc.Bacc(None, target_bir_lowering=False)

    xnT_d = nc.dram_tensor("xnT", [D, N], bf, kind="ExternalInput")
    wnames = ["wq", "wk", "wv", "wg", "war", "wai"]
    w_d = {n: nc.dram_tensor(n, [D, CH], bf, kind="ExternalInput") for n in wnames}
    wo_d = nc.dram_tensor("wo", [CH, D], bf, kind="ExternalInput")
    outT_d = nc.dram_tensor("outT", [D, N], bf, kind="ExternalOutput")

    xnT_t = xnT_d.rearrange("(ko p) n -> p ko n", p=P)
    outT_t = outT_d.rearrange("(mo p) n -> p mo n", p=P)

    with tile.TileContext(nc) as tc, ExitStack() as ctx:
        wpool = ctx.enter_context(tc.tile_pool(name="w", bufs=1))
        xpool = ctx.enter_context(tc.tile_pool(name="x", bufs=2))
        cpool = ctx.enter_context(tc.tile_pool(name="c", bufs=2))   # cross-phase, per block
        kpool = ctx.enter_context(tc.tile_pool(name="k", bufs=2))   # cross-block (scan carries)
        scr = ctx.enter_context(tc.tile_pool(name="s", bufs=8))     # fp32 scratch
        sbb = ctx.enter_context(tc.tile_pool(name="sb", bufs=12))   # bf16 scratch
        ypool = ctx.enter_context(tc.tile_pool(name="y", bufs=2))
        obp = ctx.enter_context(tc.tile_pool(name="o", bufs=2))
        gpool = ctx.enter_context(tc.tile_pool(name="g", bufs=2))
        pproj = ctx.enter_context(tc.tile_pool(name="pp", bufs=6, space="PSUM"))
        pout = ctx.enter_context(tc.tile_pool(name="po", bufs=2, space="PSUM"))

        # DMA in consumption order on the FIFO SP queue: wq (first matmul)
        # then xb0, then the remaining proj weights, xb1, wo (out-proj only
        # runs from superstep 1).
        wsb = {}
        wsb["wq"] = wpool.tile([P, KT, CH], bf, tag="w_wq", name="w_wq")
        nc.sync.dma_start(wsb["wq"][:], w_d["wq"].rearrange("(ko p) m -> p ko m", p=P))
        xbs = [None] * NBLK
        xbs[0] = xpool.tile([P, KT, T], bf, tag="xb", name="xb_0")
        nc.sync.dma_start(xbs[0][:], xnT_t[:, :, 0:T])
        for n in ["wk", "wv", "wg", "war", "wai"]:
            t_ = wpool.tile([P, KT, CH], bf, tag=f"w_{n}")
            nc.sync.dma_start(t_[:], w_d[n].rearrange("(ko p) m -> p ko m", p=P))
            wsb[n] = t_
        xbs[1] = xpool.tile([P, KT, T], bf, tag="xb", name="xb_1")
        nc.sync.dma_start(xbs[1][:], xnT_t[:, :, T:2 * T])
        wosb = wpool.tile([P, CH // P, D], bf, tag="w_wo")
        nc.sync.dma_start(wosb[:], wo_d.rearrange("(ko p) m -> p ko m", p=P))

        negmagic = wpool.tile([P, T], fp32, tag="negmagic", name="negmagic")
        nc.gpsimd.memset(negmagic[:], -MAGIC)

        prevThc = [None] * NCG   # [P,1] carry of the reduced Theta endpoint
        prevZr = [None] * NCG
        prevZi = [None] * NCG
        ys_all = [None] * NBLK
        gC = 0.0  # cross-super act-phase gate

        # Activation-table discipline: Square/Sign/Copy live in every set;
        # the set-specific phases are Sqrt -> {Arctan,Sin} -> {Silu,Tanh}.
        # Zero-valued [P,1] gate tiles, fed as activation bias (adds 0),
        # pin each set-specific phase after the previous one so the eager
        # Tile scheduler cannot interleave phases and thrash the 1.3us
        # table loads: 3-4 loads per 2-block superstep.

        def emit_proj(blk):
            xb = xbs[blk]
            PS = [None] * NCG
            for cg in range(NCG):
                cs = slice(cg * P, (cg + 1) * P)
                ps = {}
                for n in wnames:
                    pt = pproj.tile([P, T], fp32, tag="proj")
                    for k in range(KT):
                        nc.tensor.matmul(pt[:], wsb[n][:, k, cs], xb[:, k, :],
                                         start=(k == 0), stop=(k == KT - 1))
                    ps[n] = pt
                PS[cg] = ps
            return PS

        def emit_outproj(blk):
            ys = ys_all[blk]
            t0 = blk * T
            for mo in range(D // P):
                pso = pout.tile([P, T], fp32, tag="out")
                for cg in range(NCG):
                    nc.tensor.matmul(pso[:], wosb[:, cg, mo * P:(mo + 1) * P],
                                     ys[cg][:], start=(cg == 0), stop=(cg == NCG - 1))
                ob = obp.tile([P, T], bf, tag="ob")
                # Pool has no PSUM port; alternate the evacuation between
                # Act and DVE explicitly.
                if mo % 2 == 0:
                    nc.scalar.copy(ob[:], pso[:])
                else:
                    nc.vector.tensor_copy(ob[:], pso[:])
                nc.sync.dma_start(outT_t[:, mo, t0:t0 + T], ob[:])

        for blk in range(NBLK):
            b = blk
            if blk + 2 < NBLK:
                xbs[blk + 2] = xpool.tile([P, KT, T], bf, tag="xb",
                                          name=f"xb_{blk + 2}")
                nc.sync.dma_start(xbs[blk + 2][:],
                                  xnT_t[:, :, (blk + 2) * T:(blk + 3) * T])
            PS = {}
            PS[b] = emit_proj(b)
            if blk >= 2:
                emit_outproj(blk - 2)

            # --- phase A: set-free PSUM drains + Sqrt (sqrt set) --------
            kv = {}; qs = {}; gb = {}; sgn = {}; ratio = {}; r = {}
            if True:
                ps_all = PS[b]
                for cg in range(NCG):
                    ps = ps_all[cg]
                    sq1 = scr.tile([P, T], fp32, tag="scr")
                    nc.scalar.square(sq1[:], ps["war"][:])
                    sq2 = scr.tile([P, T], fp32, tag="scr")
                    nc.scalar.square(sq2[:], ps["wai"][:])
                    sgn[b, cg] = cpool.tile([P, T], bf, tag=f"sgn{cg}",
                                            name=f"sgn{cg}_{b}")
                    nc.scalar.sign(sgn[b, cg][:], ps["war"][:])
                    rec = scr.tile([P, T], fp32, tag="scr")
                    nc.vector.reciprocal_approx_fast(rec[:], ps["war"][:])
                    rt = scr.tile([P, T], fp32, tag="scr")
                    nc.vector.tensor_tensor(rt[:], ps["wai"][:], rec[:], OP.mult)
                    # clamp: the HW arctan table misbehaves for huge |x|
                    # (1/ar is unbounded); arctan(1e4) ~ pi/2 - 1e-4
                    ratio[b, cg] = cpool.tile([P, T], bf, tag=f"ratio{cg}",
                                              name=f"ratio{cg}_{b}")
                    nc.vector.tensor_scalar(ratio[b, cg][:], rt[:], 1e4, -1e4,
                                            OP.min, OP.max)
                    vs = sbb.tile([P, T], bf, tag="sbb", name=f"vs{cg}_{b}")
                    nc.scalar.copy(vs[:], ps["wv"][:])
                    kv[b, cg] = cpool.tile([P, T], bf, tag=f"kv{cg}",
                                           name=f"kv{cg}_{b}")
                    nc.vector.tensor_tensor(kv[b, cg][:], ps["wk"][:], vs[:],
                                            OP.mult)
                    qs[b, cg] = cpool.tile([P, T], bf, tag=f"qs{cg}",
                                           name=f"qs{cg}_{b}")
                    nc.scalar.copy(qs[b, cg][:], ps["wq"][:])
                    gb[b, cg] = cpool.tile([P, T], bf, tag=f"gb{cg}",
                                           name=f"gb{cg}_{b}")
                    nc.scalar.copy(gb[b, cg][:], ps["wg"][:])
                    r2 = scr.tile([P, T], fp32, tag="scr")
                    nc.gpsimd.tensor_tensor(r2[:], sq1[:], sq2[:], OP.add)
                    r[b, cg] = cpool.tile([P, T], bf, tag=f"r{cg}",
                                          name=f"r{cg}_{b}")
                    nc.scalar.activation(r[b, cg][:], r2[:], AF.Sqrt, bias=gC)

            # gate A: arctan/sin phase waits for the last Sqrt
            gA = gpool.tile([P, 1], fp32, tag="gA", name=f"gA_{blk}")
            nc.vector.tensor_scalar(gA[:], r[b, NCG - 1][:, 0:1], 0.0, None,
                                    OP.mult)

            # --- phase B (trig set): Arctan + cumsum/range-reduce + Sins
            ui = {}; ur = {}
            if True:
                thrs = [None] * NCG
                thcs = [None] * NCG
                for cg in range(NCG):
                    th = scr.tile([P, T], fp32, tag="scr")
                    nc.scalar.activation(th[:], ratio[b, cg][:], AF.Arctan,
                                         bias=gA[:, 0:1])
                    Th = scr.tile([P, T], fp32, tag="scr")
                    init = 0.0 if b == 0 else prevThc[cg][:, 0:1]
                    nc.vector.tensor_tensor_scan(Th[:], th[:], th[:],
                                                 init, OP.add, OP.bypass)
                    k2 = scr.tile([P, T], fp32, tag="scr")
                    nc.vector.affine_then_add(k2[:], Th[:], negmagic[:],
                                              INV2PI, MAGIC)
                    thr = scr.tile([P, T], fp32, tag="scr")
                    nc.vector.cody_waite_cascade(thr[:], Th[:], k2[:], C1, C2, C3)
                    thc = scr.tile([P, T], fp32, tag="scr")
                    nc.vector.add_range_wrap(thc[:], thr[:], PIH, PI,
                                             float(np.float32(TWO_PI)))
                    tc_ = kpool.tile([P, 1], fp32, tag=f"thc{cg}",
                                     name=f"thcar{cg}_{b}")
                    nc.vector.tensor_scalar(tc_[:], thr[:, T - 1:T], 0.0, None,
                                            OP.add)
                    prevThc[cg] = tc_
                    thrs[cg], thcs[cg] = thr, thc
                for cg in range(NCG):
                    ui[b, cg] = cpool.tile([P, T], bf, tag=f"ui{cg}",
                                           name=f"ui{cg}_{b}")
                    nc.scalar.activation(ui[b, cg][:], thrs[cg][:], AF.Sin,
                                         bias=gA[:, 0:1])
                    ur[b, cg] = cpool.tile([P, T], bf, tag=f"ur{cg}",
                                           name=f"ur{cg}_{b}")
                    nc.scalar.activation(ur[b, cg][:], thcs[cg][:], AF.Sin,
                                         bias=gA[:, 0:1])

            # gate B: silu/tanh phase waits for the last Sins
            gB = gpool.tile([P, 1], fp32, tag="gB", name=f"gB_{blk}")
            nc.vector.scalar_tensor_tensor(gB[:], ui[b, NCG - 1][:, 0:1], 0.0,
                                           ur[b, NCG - 1][:, 0:1],
                                           OP.mult, OP.mult)

            # --- phase C (silu set): Silu + Tanh (sigmoid synth), scans,
            # recombination. y = (q*silu(g)) * re is folded as qsg = q*sg so
            # the final product is one DVE op; y1(cg) is emitted after
            # cg+1's scans so DVE never waits on the Pool chain. ----------
            tms = None
            if True:
                ys = [None] * NCG
                res = [None] * NCG
                qsg = [None] * NCG

                def emit_y1(cg, b=b, ys=ys, res=res, qsg=qsg):
                    y1 = ypool.tile([P, T], bf, tag=f"y{cg}", name=f"y{cg}_{b}")
                    nc.vector.tensor_tensor(y1[:], qsg[cg][:], res[cg][:],
                                            OP.mult)
                    ys[cg] = y1

                for cg in range(NCG):
                    sg = sbb.tile([P, T], bf, tag="sbb", name=f"sg{cg}_{b}")
                    nc.scalar.activation(sg[:], gb[b, cg][:], AF.Silu,
                                         bias=gB[:, 0:1])
                    tm = sbb.tile([P, T], bf, tag="sbb", name=f"tm{cg}_{b}")
                    nc.scalar.activation(tm[:], r[b, cg][:], AF.Tanh,
                                         bias=gB[:, 0:1], scale=0.5)
                    tms = tm
                    qsg[cg] = cpool.tile([P, T], bf, tag=f"qsg{cg}",
                                         name=f"qsg{cg}_{b}")
                    nc.gpsimd.tensor_tensor(qsg[cg][:], qs[b, cg][:], sg[:],
                                            OP.mult)
                    # sigmoid(r) = 0.5*(1+tanh(r/2)); fold the sign in next
                    m = sbb.tile([P, T], bf, tag="sbb", name=f"m{cg}_{b}")
                    nc.vector.tensor_scalar(m[:], tm[:], 1.0, 0.5,
                                            OP.add, OP.mult)
                    mt = sbb.tile([P, T], bf, tag="sbb", name=f"mt{cg}_{b}")
                    nc.vector.tensor_tensor(mt[:], m[:], sgn[b, cg][:], OP.mult)
                    wr = sbb.tile([P, T], bf, tag="sbb", name=f"wr{cg}_{b}")
                    nc.vector.tensor_tensor(wr[:], kv[b, cg][:], ur[b, cg][:],
                                            OP.mult)
                    wi = sbb.tile([P, T], bf, tag="sbb", name=f"wi{cg}_{b}")
                    nc.vector.tensor_tensor(wi[:], kv[b, cg][:], ui[b, cg][:],
                                            OP.mult)
                    Zr = kpool.tile([P, T], bf, tag=f"Zr{cg}")
                    initr = 0.0 if b == 0 else prevZr[cg][:, T - 1:T]
                    nc.vector.tensor_tensor_scan(Zr[:], mt[:], wr[:], initr,
                                                 OP.mult, OP.add)
                    Zi = kpool.tile([P, T], bf, tag=f"Zi{cg}")
                    initi = 0.0 if b == 0 else prevZi[cg][:, T - 1:T]
                    nc.vector.tensor_tensor_scan(Zi[:], mt[:], wi[:], initi,
                                                 OP.mult, OP.add)
                    if cg > 0:
                        emit_y1(cg - 1)
                    t1 = sbb.tile([P, T], bf, tag="sbb", name=f"t1{cg}_{b}")
                    nc.gpsimd.tensor_tensor(t1[:], ur[b, cg][:], Zr[:], OP.mult)
                    t2 = sbb.tile([P, T], bf, tag="sbb", name=f"t2{cg}_{b}")
                    nc.gpsimd.tensor_tensor(t2[:], ui[b, cg][:], Zi[:], OP.mult)
                    re = sbb.tile([P, T], bf, tag="sbb", name=f"re{cg}_{b}")
                    nc.gpsimd.tensor_tensor(re[:], t1[:], t2[:], OP.add)
                    res[cg] = re
                    prevZr[cg], prevZi[cg] = Zr, Zi
                emit_y1(NCG - 1)
                ys_all[b] = ys

            # gate C: next superstep's Sqrt waits for the last Tanh
            gCt = gpool.tile([P, 1], fp32, tag="gC", name=f"gC_{blk}")
            nc.vector.tensor_scalar(gCt[:], tms[:, 0:1], 0.0, None, OP.mult)
            gC = gCt[:, 0:1]

        emit_outproj(NBLK - 2)
        emit_outproj(NBLK - 1)

    nc.finalize()
    return nc


def _get_nc():
    global _NC
    if _NC is None:
        _NC = _build()
    return _NC


def kernel(**inputs):
    global LAST_RESULT
    from concourse.bass_utils import run_bass_kernel_spmd

    x = np.asarray(inputs["x"], np.float32)
    gamma = np.asarray(inputs["gamma"], np.float32)
    wq = np.asarray(inputs["wq"], np.float32)
    wk = np.asarray(inputs["wk"], np.float32)
    wv = np.asarray(inputs["wv"], np.float32)
    wa = np.asarray(inputs["wa"], np.float32)
    wg = np.asarray(inputs["wg"], np.float32)
    wo = np.asarray(inputs["wo"], np.float32)

    inv = 1.0 / np.sqrt((x * x).sum(-1, keepdims=True) + np.float32(EPS))
    xn = (inv * x * gamma * np.float32(math.sqrt(D))).astype(np.float32)
    xnT = np.ascontiguousarray(xn.transpose(0, 2, 1)).astype(BF16)  # (B, D, N)

    in_maps = []
    for core in range(8):
        b, h = core // 2, core % 2
        ch = slice(h * CH, (h + 1) * CH)
        in_maps.append({
            "xnT": xnT[b],
            "wq": np.ascontiguousarray(wq[:, ch]).astype(BF16),
            "wk": np.ascontiguousarray(wk[:, ch]).astype(BF16),
            "wv": np.ascontiguousarray(wv[:, ch]).astype(BF16),
            "wg": np.ascontiguousarray(wg[:, ch]).astype(BF16),
            "war": np.ascontiguousarray(wa[:, h * CH:(h + 1) * CH]).astype(BF16),
            "wai": np.ascontiguousarray(wa[:, D + h * CH:D + (h + 1) * CH]).astype(BF16),
            "wo": np.ascontiguousarray(wo[ch, :]).astype(BF16),
        })

    nc = _get_nc()
    trace = bool(int(os.environ.get("GATELOOP_TRACE", "0")))
    LAST_RESULT = run_bass_kernel_spmd(
        nc, in_maps, core_ids=list(range(8)), trace=trace,
        trace_cores=list(range(8)) if trace else None,
    )
    res = LAST_RESULT.results

    out = np.empty((B, N, D), np.float32)
    for b in range(B):
        acc = (res[2 * b]["outT"].astype(np.float32)
               + res[2 * b + 1]["outT"].astype(np.float32))   # (D, N)
        out[b] = acc.T
    return out


# revision 11
# speedup vs baseline: 1.2031x; 1.0492x over previous
"""GateLoop (B=4, N=4096, D=1024) Trainium2 kernel over 8 NeuronCores.

Sharding: data-parallel over the 4 batch elements x 2-way tensor-parallel
split of the D=1024 recurrence channels (the complex diagonal recurrence is
independent per channel). Core c handles batch c//2, channels
[(c%2)*512 : (c%2+1)*512]. Each core computes its projections, runs the
scan over the full sequence for its 512 channels, and produces a partial
y @ wo[ch, :] of shape (1024, 4096) (transposed). The host sums the two
partials per batch and transposes back. No cross-core communication.

Scan formulation (avoids complex arithmetic + overflow): with
a_t = m_t * cis(phi_t), m_t = sigmoid(|a_t|), theta_t = arctan(ai/ar)
in (-pi/2, pi/2) (SIGNED division so the ar<0 half-plane flip folds into
the signed multiplier mt_t = m_t * sign(ar_t)). With Theta_t =
cumsum(theta) the recurrence becomes two independent REAL first-order
scans
    Zr_t = mt_t * Zr_{t-1} + kv_t * cos(Theta_t)
    Zi_t = mt_t * Zi_{t-1} + kv_t * sin(Theta_t)
and Re(S_t) = cos(Theta_t) * Zr_t + sin(Theta_t) * Zi_t, which map onto
the DVE TensorTensorScan instruction (fp32 state, |mt| < 1 so stable).
The Theta scan re-bases each block from the range-reduced thr endpoint
(equivalent mod 2pi, keeps Theta < ~810 in fp32); sigmoid is synthesized
as 0.5*(1+tanh(r/2)) so it lands in the silu table set.

Schedule: per 512-token window, the Act table rotation is
  [sqrt set]  drains of block b + Sqrt(b)
  [trig set]  Arctan(b) + DVE cumsum/range-reduce + Sins(b)
  [silu set]  Silu/Tanh of block b-1 (STALE data -> no waiting), then the
              b-1 scans and products on DVE/Pool.
Deferring phase C by one block takes the cody->sin->silu serialization off
the cross-block gate chain (gC fires ~immediately), so the 48 us of PE
matmuls per window pace the kernel instead of the elementwise chain.
Out-projection of block j runs as a 32-matmul burst after proj(j+2).
Projections for (q,g), (k,v), (ar,ai) land in three 2-bank PSUM tiles per
cg so each pair drains with ONE wide Act op; ratio=clamp(ai/ar) is a
single custom DVE op (RT_CLAMP); k*v and q*silu(g) run on Pool.
Weights DMA in consumption order (wq, xb0, wk, wv, wg, war, wai, xb1, wo)
on the FIFO SP queue.
"""
import math
import os

import numpy as np
import ml_dtypes

B, N, D = 4, 4096, 1024
CH = 512            # channels per core (tensor-parallel half)
NCG = CH // 128     # 4 channel groups of 128 partitions
T = 512             # token block
NBLK = N // T
P = 128
KT = D // P         # contraction tiles
EPS = 1e-5
BF16 = ml_dtypes.bfloat16

TWO_PI = 2 * math.pi
C1 = float(np.float32(6.28125))
C2 = float(np.float32(np.float64(TWO_PI) - 6.28125))
C3 = float(np.float32(np.float64(TWO_PI) - 6.28125
                      - np.float64(np.float32(np.float64(TWO_PI) - 6.28125))))
MAGIC = float(np.float32(1.5 * 2 ** 23))
INV2PI = float(np.float32(1.0 / TWO_PI))
PI = float(np.float32(math.pi))
PIH = float(np.float32(math.pi / 2))
RCLAMP = 1e4

_NC = None
LAST_RESULT = None  # BassKernelResults of the most recent run (for profiling)
_RT_CLAMP = None


def _get_rt_clamp():
    """Register (once) a custom DVE op: out = min(max(in0*in1, s1), s0).

    Fuses the ratio multiply (PSUM ai x SBUF 1/ar) with the arctan-domain
    clamp; 3 uop stages. Registered by appending to concourse.dve_ops.OPS
    with the sha pinned from a local lower() pass.
    """
    global _RT_CLAMP
    if _RT_CLAMP is not None:
        return _RT_CLAMP
    import concourse.dve_ops as dve_ops
    from concourse.dve_ops import DveOp
    from concourse.dve_spec import Spec, Src0, Src1, C0 as SC0, C1 as SC1, \
        lower, minn, maxx, _has_src1
    from concourse.dve_uop import DveOpSpec
    from concourse.dve_table_gen import dve_ver_for

    name = "RT_CLAMP_GL"
    if name in dve_ops._SUB_OPCODE_FOR_NAME:
        _RT_CLAMP = next(op for op in dve_ops.OPS if op.name == name)
        return _RT_CLAMP
    spec = Spec(
        body=minn(maxx(Src0 * Src1, SC1), SC0),
        reference=lambda in0, in1, s0, s1, imm2: np.minimum(
            np.maximum(in0.astype(np.float32) * in1, s1), s0
        ).astype(np.float32),
    )
    row = dve_ops._CUSTOM_DVE_ROW_BASE + len(dve_ops.OPS)
    dve_ops._SUB_OPCODE_FOR_NAME[name] = row
    shas = {}
    for ver in ("v3", "v4"):
        uops = lower(spec, ver=ver)
        shas[ver] = DveOpSpec(name=name, opcode=row, uops=uops,
                              rd1_en=_has_src1(spec)).sha(ver)
    op = DveOp(name, spec, subdim=False, uops_sha=shas)
    dve_ops.OPS.append(op)
    dve_ops.CUSTOM_DVE_SPECS[name] = spec
    _RT_CLAMP = op
    return op


def _build():
    from contextlib import ExitStack
    from concourse import bacc
    import concourse.mybir as mybir
    import concourse.tile as tile
    from concourse.mybir import ActivationFunctionType as AF, AluOpType as OP

    fp32 = mybir.dt.float32
    bf = mybir.dt.bfloat16
    rt_clamp = _get_rt_clamp()

    nc = bacc.Bacc(None, target_bir_lowering=False)

    xnT_d = nc.dram_tensor("xnT", [D, N], bf, kind="ExternalInput")
    wnames = ["wq", "wk", "wv", "wg", "war", "wai"]
    w_d = {n: nc.dram_tensor(n, [D, CH], bf, kind="ExternalInput") for n in wnames}
    wo_d = nc.dram_tensor("wo", [CH, D], bf, kind="ExternalInput")
    outT_d = nc.dram_tensor("outT", [D, N], bf, kind="ExternalOutput")

    xnT_t = xnT_d.rearrange("(ko p) n -> p ko n", p=P)
    outT_t = outT_d.rearrange("(mo p) n -> p mo n", p=P)

    with tile.TileContext(nc) as tc, ExitStack() as ctx:
        wpool = ctx.enter_context(tc.tile_pool(name="w", bufs=1))
        xpool = ctx.enter_context(tc.tile_pool(name="x", bufs=2))
        cpool = ctx.enter_context(tc.tile_pool(name="c", bufs=2))   # cross-phase, per block
        kpool = ctx.enter_context(tc.tile_pool(name="k", bufs=2))   # cross-block (scan carries)
        scr = ctx.enter_context(tc.tile_pool(name="s", bufs=6))     # fp32 [P,T] scratch
        sc2 = ctx.enter_context(tc.tile_pool(name="s2", bufs=2))    # fp32 [P,2T] scratch
        sbb = ctx.enter_context(tc.tile_pool(name="sb", bufs=11))   # bf16 scratch
        kvp = ctx.enter_context(tc.tile_pool(name="kv2", bufs=2))   # bf16 [P,2T] k|v drain
        ypool = ctx.enter_context(tc.tile_pool(name="y", bufs=2))
        obp = ctx.enter_context(tc.tile_pool(name="o", bufs=2))
        gpool = ctx.enter_context(tc.tile_pool(name="g", bufs=2))
        pproj = ctx.enter_context(tc.tile_pool(name="pp", bufs=3, space="PSUM"))
        pout = ctx.enter_context(tc.tile_pool(name="po", bufs=2, space="PSUM"))

        # DMA in consumption order on the FIFO SP queue.
        wsb = {}
        wsb["wq"] = wpool.tile([P, KT, CH], bf, tag="w_wq", name="w_wq")
        nc.sync.dma_start(wsb["wq"][:], w_d["wq"].rearrange("(ko p) m -> p ko m", p=P))
        xbs = [None] * NBLK
        xbs[0] = xpool.tile([P, KT, T], bf, tag="xb", name="xb_0")
        nc.sync.dma_start(xbs[0][:], xnT_t[:, :, 0:T])
        for n in ["wg", "wk", "wv", "war", "wai"]:
            t_ = wpool.tile([P, KT, CH], bf, tag=f"w_{n}")
            nc.sync.dma_start(t_[:], w_d[n].rearrange("(ko p) m -> p ko m", p=P))
            wsb[n] = t_
        xbs[1] = xpool.tile([P, KT, T], bf, tag="xb", name="xb_1")
        nc.sync.dma_start(xbs[1][:], xnT_t[:, :, T:2 * T])
        wosb = wpool.tile([P, CH // P, D], bf, tag="w_wo")
        nc.sync.dma_start(wosb[:], wo_d.rearrange("(ko p) m -> p ko m", p=P))

        negmagic = wpool.tile([P, T], fp32, tag="negmagic", name="negmagic")
        nc.gpsimd.memset(negmagic[:], -MAGIC)

        prevThc = [None] * NCG   # [P,1] carry of the reduced Theta endpoint
        prevZr = [None] * NCG
        prevZi = [None] * NCG
        ys_all = [None] * NBLK
        gC = 0.0  # gate opening the sqrt set each window

        # PSUM pairing per cg: three 2-bank tiles (q|g), (k|v), (ar|ai);
        # each half is its own 8-matmul accumulation group, and each pair
        # drains with one wide Act op.
        PAIRS = [("wq", "wg"), ("wk", "wv"), ("war", "wai")]

        def emit_proj(blk):
            xb = xbs[blk]
            PS = [None] * NCG
            for cg in range(NCG):
                cs = slice(cg * P, (cg + 1) * P)
                ps = {}
                for n0, n1 in PAIRS:
                    pt = pproj.tile([P, 2, T], fp32, tag="proj")
                    for h, n in ((0, n0), (1, n1)):
                        for k in range(KT):
                            nc.tensor.matmul(pt[:, h, :], wsb[n][:, k, cs],
                                             xb[:, k, :],
                                             start=(k == 0), stop=(k == KT - 1))
                    ps[n0, n1] = pt
                PS[cg] = ps
            return PS

        def emit_outproj(blk):
            ys = ys_all[blk]
            t0 = blk * T
            for mo in range(D // P):
                pso = pout.tile([P, T], fp32, tag="out")
                for cg in range(NCG):
                    nc.tensor.matmul(pso[:], wosb[:, cg, mo * P:(mo + 1) * P],
                                     ys[cg][:], start=(cg == 0), stop=(cg == NCG - 1))
                ob = obp.tile([P, T], bf, tag="ob")
                # Pool has no PSUM port; alternate the evacuation between
                # Act and DVE explicitly.
                if mo % 2 == 0:
                    nc.scalar.copy(ob[:], pso[:])
                else:
                    nc.vector.tensor_copy(ob[:], pso[:])
                nc.sync.dma_start(outT_t[:, mo, t0:t0 + T], ob[:])

        prev = None  # phase-C inputs of the previous block

        def emit_phase_c(b, pv, gB):
            """Silu/Tanh + scans + products for block b (data from `pv`),
            emitted one window after block b's projections."""
            ys = [None] * NCG
            res = [None] * NCG
            qsg = [None] * NCG

            def emit_y1(cg):
                y1 = ypool.tile([P, T], bf, tag=f"y{cg}", name=f"y{cg}_{b}")
                nc.vector.tensor_tensor(y1[:], qsg[cg][:], res[cg][:], OP.mult)
                ys[cg] = y1

            tms = None
            for cg in range(NCG):
                sg = sbb.tile([P, T], bf, tag="sbb", name=f"sg{cg}_{b}")
                nc.scalar.activation(sg[:], pv["qg", cg][:, 1, :], AF.Silu,
                                     bias=gB[:, 0:1])
                tm = sbb.tile([P, T], bf, tag="sbb", name=f"tm{cg}_{b}")
                nc.scalar.activation(tm[:], pv["r", cg][:], AF.Tanh,
                                     bias=gB[:, 0:1], scale=0.5)
                tms = tm
                qsg[cg] = cpool.tile([P, T], bf, tag=f"qsg{cg}",
                                     name=f"qsg{cg}_{b}")
                nc.gpsimd.tensor_tensor(qsg[cg][:], pv["qg", cg][:, 0, :], sg[:],
                                        OP.mult)
                # sigmoid(r) = 0.5*(1+tanh(r/2)), sign folded via sgn
                m = sbb.tile([P, T], bf, tag="sbb", name=f"m{cg}_{b}")
                nc.vector.tensor_scalar(m[:], tm[:], 1.0, 0.5, OP.add, OP.mult)
                mt = sbb.tile([P, T], bf, tag="sbb", name=f"mt{cg}_{b}")
                nc.vector.tensor_tensor(mt[:], m[:], pv["sgn", cg][:], OP.mult)
                wr = sbb.tile([P, T], bf, tag="sbb", name=f"wr{cg}_{b}")
                nc.vector.tensor_tensor(wr[:], pv["kv", cg][:], pv["ur", cg][:],
                                        OP.mult)
                wi = sbb.tile([P, T], bf, tag="sbb", name=f"wi{cg}_{b}")
                nc.vector.tensor_tensor(wi[:], pv["kv", cg][:], pv["ui", cg][:],
                                        OP.mult)
                Zr = kpool.tile([P, T], bf, tag=f"Zr{cg}")
                initr = 0.0 if b == 0 else prevZr[cg][:, T - 1:T]
                nc.vector.tensor_tensor_scan(Zr[:], mt[:], wr[:], initr,
                                             OP.mult, OP.add)
                Zi = kpool.tile([P, T], bf, tag=f"Zi{cg}")
                initi = 0.0 if b == 0 else prevZi[cg][:, T - 1:T]
                nc.vector.tensor_tensor_scan(Zi[:], mt[:], wi[:], initi,
                                             OP.mult, OP.add)
                if cg > 0:
                    emit_y1(cg - 1)
                t1 = sbb.tile([P, T], bf, tag="sbb", name=f"t1{cg}_{b}")
                nc.gpsimd.tensor_tensor(t1[:], pv["ur", cg][:], Zr[:], OP.mult)
                t2 = sbb.tile([P, T], bf, tag="sbb", name=f"t2{cg}_{b}")
                nc.gpsimd.tensor_tensor(t2[:], pv["ui", cg][:], Zi[:], OP.mult)
                re = sbb.tile([P, T], bf, tag="sbb", name=f"re{cg}_{b}")
                nc.gpsimd.tensor_tensor(re[:], t1[:], t2[:], OP.add)
                res[cg] = re
                prevZr[cg], prevZi[cg] = Zr, Zi
            emit_y1(NCG - 1)
            ys_all[b] = ys
            return tms

        for blk in range(NBLK):
            b = blk
            if blk + 2 < NBLK:
                xbs[blk + 2] = xpool.tile([P, KT, T], bf, tag="xb",
                                          name=f"xb_{blk + 2}")
                nc.sync.dma_start(xbs[blk + 2][:],
                                  xnT_t[:, :, (blk + 2) * T:(blk + 3) * T])
            PS = emit_proj(b)
            if blk >= 2:
                emit_outproj(blk - 2)

            # --- phase A: paired PSUM drains + Sqrt (sqrt set) ----------
            cur = {}
            r2s = [None] * NCG
            for cg in range(NCG):
                ps_qg = PS[cg]["wq", "wg"]
                ps_kv = PS[cg]["wk", "wv"]
                ps_a = PS[cg]["war", "wai"]
                qg = cpool.tile([P, 2, T], bf, tag=f"qg{cg}", name=f"qg{cg}_{b}")
                nc.scalar.copy(qg[:], ps_qg[:])
                kvd = kvp.tile([P, 2, T], bf, tag="kvd", name=f"kvd{cg}_{b}")
                nc.scalar.copy(kvd[:], ps_kv[:])
                kv = cpool.tile([P, T], bf, tag=f"kv{cg}", name=f"kv{cg}_{b}")
                nc.gpsimd.tensor_tensor(kv[:], kvd[:, 0, :], kvd[:, 1, :],
                                        OP.mult)
                sq = sc2.tile([P, 2, T], fp32, tag="sc2")
                nc.scalar.square(sq[:], ps_a[:])
                sgn = cpool.tile([P, T], bf, tag=f"sgn{cg}", name=f"sgn{cg}_{b}")
                nc.scalar.sign(sgn[:], ps_a[:, 0, :])
                rec = scr.tile([P, T], fp32, tag="scr")
                nc.vector.reciprocal_approx_fast(rec[:], ps_a[:, 0, :])
                # ratio = clamp(ai * (1/ar)) in ONE custom DVE op; the HW
                # arctan table misbehaves for huge |x|.
                ratio = cpool.tile([P, T], bf, tag=f"ratio{cg}",
                                   name=f"ratio{cg}_{b}")
                nc.vector._custom_dve(rt_clamp, out=ratio[:], in0=ps_a[:, 1, :],
                                      in1=rec[:], s0=RCLAMP, s1=-RCLAMP)
                r2 = scr.tile([P, T], fp32, tag="scr")
                nc.gpsimd.tensor_tensor(r2[:], sq[:, 0, :], sq[:, 1, :], OP.add)
                r2s[cg] = r2
                cur["qg", cg] = qg
                cur["kv", cg] = kv
                cur["sgn", cg] = sgn
                cur["ratio", cg] = ratio
            for cg in range(NCG):
                r = cpool.tile([P, T], bf, tag=f"r{cg}", name=f"r{cg}_{b}")
                nc.scalar.activation(r[:], r2s[cg][:], AF.Sqrt, bias=gC)
                cur["r", cg] = r

            # gate A: the trig phase waits for the last Sqrt
            gA = gpool.tile([P, 1], fp32, tag="gA", name=f"gA_{blk}")
            nc.vector.tensor_scalar(gA[:], cur["r", NCG - 1][:, 0:1], 0.0, None,
                                    OP.mult)

            # --- phase B (trig set): Arctan + cumsum/range-reduce + Sins
            thrs = [None] * NCG
            thcs = [None] * NCG
            for cg in range(NCG):
                th = scr.tile([P, T], fp32, tag="scr")
                nc.scalar.activation(th[:], cur["ratio", cg][:], AF.Arctan,
                                     bias=gA[:, 0:1])
                Th = scr.tile([P, T], fp32, tag="scr")
                init = 0.0 if b == 0 else prevThc[cg][:, 0:1]
                nc.vector.tensor_tensor_scan(Th[:], th[:], th[:],
                                             init, OP.add, OP.bypass)
                k2 = scr.tile([P, T], fp32, tag="scr")
                nc.vector.affine_then_add(k2[:], Th[:], negmagic[:],
                                          INV2PI, MAGIC)
                thr = scr.tile([P, T], fp32, tag="scr")
                nc.vector.cody_waite_cascade(thr[:], Th[:], k2[:], C1, C2, C3)
                thc = scr.tile([P, T], fp32, tag="scr")
                nc.vector.add_range_wrap(thc[:], thr[:], PIH, PI,
                                         float(np.float32(TWO_PI)))
                tc_ = kpool.tile([P, 1], fp32, tag=f"thc{cg}",
                                 name=f"thcar{cg}_{b}")
                nc.vector.tensor_scalar(tc_[:], thr[:, T - 1:T], 0.0, None,
                                        OP.add)
                prevThc[cg] = tc_
                thrs[cg], thcs[cg] = thr, thc
            for cg in range(NCG):
                uit = cpool.tile([P, T], bf, tag=f"ui{cg}", name=f"ui{cg}_{b}")
                nc.scalar.activation(uit[:], thrs[cg][:], AF.Sin, bias=gA[:, 0:1])
                urt = cpool.tile([P, T], bf, tag=f"ur{cg}", name=f"ur{cg}_{b}")
                nc.scalar.activation(urt[:], thcs[cg][:], AF.Sin, bias=gA[:, 0:1])
                cur["ui", cg] = uit
                cur["ur", cg] = urt

            # gate B: the silu phase waits for the last Sins
            gB = gpool.tile([P, 1], fp32, tag="gB", name=f"gB_{blk}")
            nc.vector.scalar_tensor_tensor(gB[:], cur["ui", NCG - 1][:, 0:1],
                                           0.0, cur["ur", NCG - 1][:, 0:1],
                                           OP.mult, OP.mult)

            # --- phase C (silu set): previous block's Silu/Tanh + scans --
            if prev is not None:
                tms = emit_phase_c(b - 1, prev, gB)
                gCt = gpool.tile([P, 1], fp32, tag="gC", name=f"gC_{blk}")
                nc.vector.tensor_scalar(gCt[:], tms[:, 0:1], 0.0, None, OP.mult)
            else:
                gCt = gpool.tile([P, 1], fp32, tag="gC", name=f"gC_{blk}")
                nc.vector.tensor_scalar(gCt[:], gB[:, 0:1], 0.0, None, OP.add)
            gC = gCt[:, 0:1]

            prev = cur
            lastgB = gB

        # tail: phase C of the final block, then the last two out-projs
        emit_phase_c(NBLK - 1, prev, lastgB)
        emit_outproj(NBLK - 2)
        emit_outproj(NBLK - 1)

    nc.finalize()
    return nc


def _get_nc():
    global _NC
    if _NC is None:
        _NC = _build()
    return _NC


def kernel(**inputs):
    global LAST_RESULT
    from concourse.bass_utils import run_bass_kernel_spmd

    x = np.asarray(inputs["x"], np.float32)
    gamma = np.asarray(inputs["gamma"], np.float32)
    wq = np.asarray(inputs["wq"], np.float32)
    wk = np.asarray(inputs["wk"], np.float32)
    wv = np.asarray(inputs["wv"], np.float32)
    wa = np.asarray(inputs["wa"], np.float32)
    wg = np.asarray(inputs["wg"], np.float32)
    wo = np.asarray(inputs["wo"], np.float32)

    inv = 1.0 / np.sqrt((x * x).sum(-1, keepdims=True) + np.float32(EPS))
    xn = (inv * x * gamma * np.float32(math.sqrt(D))).astype(np.float32)
    xnT = np.ascontiguousarray(xn.transpose(0, 2, 1)).astype(BF16)  # (B, D, N)

    in_maps = []
    for core in range(8):
        b, h = core // 2, core % 2
        ch = slice(h * CH, (h + 1) * CH)
        in_maps.append({
            "xnT": xnT[b],
            "wq": np.ascontiguousarray(wq[:, ch]).astype(BF16),
            "wk": np.ascontiguousarray(wk[:, ch]).astype(BF16),
            "wv": np.ascontiguousarray(wv[:, ch]).astype(BF16),
            "wg": np.ascontiguousarray(wg[:, ch]).astype(BF16),
            "war": np.ascontiguousarray(wa[:, h * CH:(h + 1) * CH]).astype(BF16),
            "wai": np.ascontiguousarray(wa[:, D + h * CH:D + (h + 1) * CH]).astype(BF16),
            "wo": np.ascontiguousarray(wo[ch, :]).astype(BF16),
        })

    nc = _get_nc()
    trace = bool(int(os.environ.get("GATELOOP_TRACE", "0")))
    LAST_RESULT = run_bass_kernel_spmd(
        nc, in_maps, core_ids=list(range(8)), trace=trace,
        trace_cores=list(range(8)) if trace else None,
    )
    res = LAST_RESULT.results

    out = np.empty((B, N, D), np.float32)
    for b in range(B):
        acc = (res[2 * b]["outT"].astype(np.float32)
               + res[2 * b + 1]["outT"].astype(np.float32))   # (D, N)
        out[b] = acc.T
    return out
